# revision 15
# baseline (speedup 1.0000x reference)
"""Trainium2 Bass kernel for fused attention block (B=2, S=2048, H=1024, N=16, D=64).

Sharding: 8 cores = 2 batches (DP) x 4 head-groups (TP, 4 heads each).
Each core computes q/k/v projections + LN + RoPE + attention for its 4 heads,
AllGathers normalized attention outputs (bf16) within its batch quad (split in
two so the first gather overlaps attention), then computes a 256-column slice
of the output projection.
"""

import numpy as np
import ml_dtypes

import concourse.bass as bass
from concourse import bacc
import concourse.mybir as mybir
import concourse.tile as tile
from concourse.masks import make_identity

# problem shape (hardcoded per contract)
B, S, H, NH, D = 2, 2048, 1024, 16, 64
EPS = 1.0 / 65530.0
NCORES = 8
HPC = 4            # heads per core
OC = HPC * D       # 256 head-dims per core
P = 128
SB = S // P        # 16 s-blocks
KC = H // P        # 8 contraction chunks of 128
D2 = D // 2
SCALE = 1.0 / 8.0  # 1/sqrt(D)
DV = D + 1         # V columns per head incl. ones column
SC = 512           # s-chunk for PV accumulation
NSC = S // SC      # 4

BF = mybir.dt.bfloat16
F32 = mybir.dt.float32
ALU = mybir.AluOpType
ACTF = mybir.ActivationFunctionType

QK_N = 1024        # moving free dim for QK matmuls (bf16 allows 1024)


def build_nc():
    nc = bacc.Bacc(num_devices=NCORES)

    hT = nc.declare_dram_parameter("hT", [H, S], BF, isOutput=False)
    qwT = nc.declare_dram_parameter("qwT", [H, OC], BF, isOutput=False)
    kwT = nc.declare_dram_parameter("kwT", [H, OC], BF, isOutput=False)
    vwT = nc.declare_dram_parameter("vwT", [H, OC], BF, isOutput=False)
    owT = nc.declare_dram_parameter("owT", [H, OC], BF, isOutput=False)
    qb = nc.declare_dram_parameter("qb", [P, OC], F32, isOutput=False)
    kb = nc.declare_dram_parameter("kb", [P, OC], F32, isOutput=False)
    vb = nc.declare_dram_parameter("vb", [P, OC], F32, isOutput=False)
    ob = nc.declare_dram_parameter("ob", [P, OC], F32, isOutput=False)
    cosd = nc.declare_dram_parameter("cosd", [S, D], F32, isOutput=False)
    sind = nc.declare_dram_parameter("sind", [S, D], F32, isOutput=False)
    out = nc.declare_dram_parameter("out", [S, OC], F32, isOutput=True)

    with tile.TileContext(nc) as tc:
        with tc.tile_pool(name="persist", bufs=1) as persist:
            ident = persist.tile([P, P], BF)
            make_identity(nc, ident)
            cos_sb = persist.tile([P, SB, D], F32)
            nc.sync.dma_start(cos_sb[:], cosd[:].rearrange("(a p) d -> p a d", p=P))
            sin_sb = persist.tile([P, SB, D], F32)
            nc.sync.dma_start(sin_sb[:], sind[:].rearrange("(a p) d -> p a d", p=P))
            qb_sb = persist.tile([P, OC], F32)
            nc.sync.dma_start(qb_sb[:], qb[:])
            kb_sb = persist.tile([P, OC], F32)
            nc.sync.dma_start(kb_sb[:], kb[:])
            vb_sb = persist.tile([P, OC], F32)
            nc.sync.dma_start(vb_sb[:], vb[:])
            ob_sb = persist.tile([P, OC], F32)
            nc.sync.dma_start(ob_sb[:], ob[:])

            # q/k transposed per head, zero-padded contraction rows 64..127
            qTp = persist.tile([P, HPC, S], BF)
            kTp = persist.tile([P, HPC, S], BF)
            nc.gpsimd.memset(qTp[:], 0.0)
            nc.gpsimd.memset(kTp[:], 0.0)
            # v in [s, head*(D+1)] layout: 64 data cols + 1 ones col per head
            Vp = persist.tile([P, SB, HPC * DV], BF)
            for h in range(HPC):
                nc.gpsimd.memset(Vp[:, :, h * DV + D : (h + 1) * DV], 1.0)
            attnT = persist.tile([D, HPC, S], BF)      # normalized [d, h, s]

            mu_q = persist.tile([P, SB, HPC], F32)
            mu_k = persist.tile([P, SB, HPC], F32)
            var_q = persist.tile([P, SB, HPC], F32)
            var_k = persist.tile([P, SB, HPC], F32)
            rstd_q = persist.tile([P, SB, HPC], F32)
            rstd_k = persist.tile([P, SB, HPC], F32)

            # ---------------- phase P: projections + LN stats -------------
            with tc.tile_pool(name="pw", bufs=1) as pw, \
                 tc.tile_pool(name="ppsum", bufs=2, space="PSUM") as ppsum, \
                 tc.tile_pool(name="ptmp", bufs=3) as ptmp, \
                 tc.tile_pool(name="tpsum", bufs=2, space="PSUM") as tpsum:
                hT_sb = pw.tile([P, KC, S], BF)
                nc.sync.dma_start(hT_sb[:], hT[:].rearrange("(a p) s -> p a s", p=P))
                qwT_sb = pw.tile([P, KC, OC], BF)
                nc.sync.dma_start(qwT_sb[:], qwT[:].rearrange("(a p) o -> p a o", p=P))
                kwT_sb = pw.tile([P, KC, OC], BF)
                nc.sync.dma_start(kwT_sb[:], kwT[:].rearrange("(a p) o -> p a o", p=P))
                vwT_sb = pw.tile([P, KC, OC], BF)
                nc.sync.dma_start(vwT_sb[:], vwT[:].rearrange("(a p) o -> p a o", p=P))
                qf = pw.tile([P, SB, OC], F32)
                kf = pw.tile([P, SB, OC], F32)

                for sb in range(SB):
                    psq = ppsum.tile([P, OC], F32, name=f"psq{sb}", tag="psq")
                    psk = ppsum.tile([P, OC], F32, name=f"psk{sb}", tag="psk")
                    psv = ppsum.tile([P, OC], F32, name=f"psv{sb}", tag="psv")
                    for kc in range(KC):
                        st, sp = kc == 0, kc == KC - 1
                        lhs = hT_sb[:, kc, sb * P : (sb + 1) * P]
                        nc.tensor.matmul(psq[:], lhs, qwT_sb[:, kc], start=st, stop=sp)
                        nc.tensor.matmul(psk[:], lhs, kwT_sb[:, kc], start=st, stop=sp)
                        nc.tensor.matmul(psv[:], lhs, vwT_sb[:, kc], start=st, stop=sp)
                    nc.vector.tensor_tensor(out=qf[:, sb], in0=psq[:], in1=qb_sb[:], op=ALU.add)
                    nc.vector.tensor_tensor(out=kf[:, sb], in0=psk[:], in1=kb_sb[:], op=ALU.add)
                    # v + bias -> Vp data columns (strided around ones cols)
                    nc.vector.tensor_tensor(
                        out=Vp[:, sb].rearrange("p (h e) -> p h e", h=HPC)[:, :, 0:D],
                        in0=psv[:].rearrange("p (h d) -> p h d", h=HPC),
                        in1=vb_sb[:].rearrange("p (h d) -> p h d", h=HPC),
                        op=ALU.add,
                    )
                    # LN stats: mean and mean-of-square per (s, head)
                    for xf, mu, var in ((qf, mu_q, var_q), (kf, mu_k, var_k)):
                        xv = xf[:, sb].rearrange("p (h d) -> p h d", h=HPC)
                        sq = ptmp.tile([P, HPC, D], F32, name=f"sq{sb}", tag="sq")
                        nc.vector.tensor_tensor(out=sq[:], in0=xv, in1=xv, op=ALU.mult)
                        nc.vector.tensor_reduce(
                            out=mu[:, sb], in_=xv, axis=mybir.AxisListType.X, op=ALU.add
                        )
                        nc.vector.tensor_reduce(
                            out=var[:, sb], in_=sq[:], axis=mybir.AxisListType.X, op=ALU.add
                        )
                        nc.vector.tensor_scalar_mul(mu[:, sb], mu[:, sb], 1.0 / D)
                        nc.vector.tensor_scalar_mul(var[:, sb], var[:, sb], 1.0 / D)
                        mu2 = ptmp.tile([P, HPC], F32, name=f"mu2{sb}", tag="mu2")
                        nc.vector.tensor_tensor(out=mu2[:], in0=mu[:, sb], in1=mu[:, sb], op=ALU.mult)
                        nc.vector.tensor_tensor(out=var[:, sb], in0=var[:, sb], in1=mu2[:], op=ALU.subtract)

                # rstd = 1/sqrt(var + eps); fold attention scale into q's rstd
                std_q = persist.tile([P, SB, HPC], F32)
                std_k = persist.tile([P, SB, HPC], F32)
                eps_t = persist.tile([P, 1], F32)
                nc.gpsimd.memset(eps_t[:], EPS)
                nc.scalar.activation(std_q[:], var_q[:], ACTF.Sqrt, bias=eps_t[:])
                nc.scalar.activation(std_k[:], var_k[:], ACTF.Sqrt, bias=eps_t[:])
                nc.vector.reciprocal(rstd_q[:], std_q[:])
                nc.vector.reciprocal(rstd_k[:], std_k[:])
                nc.vector.tensor_scalar_mul(rstd_q[:], rstd_q[:], SCALE)

                # ---------------- phase L: LN apply + rope (batched) ------
                for xf, mu, rstd, xTp in (
                    (qf, mu_q, rstd_q, qTp),
                    (kf, mu_k, rstd_k, kTp),
                ):
                    xv = xf[:].rearrange("p s (h d) -> p s h d", h=HPC)
                    mu_b = mu[:, :, :, None].to_broadcast((P, SB, HPC, D))
                    rs_b = rstd[:, :, :, None].to_broadcast((P, SB, HPC, D))
                    nc.vector.tensor_tensor(out=xv, in0=xv, in1=mu_b, op=ALU.subtract)
                    nc.vector.tensor_tensor(out=xv, in0=xv, in1=rs_b, op=ALU.mult)
                    # rope
                    cb = cos_sb[:, :, None, :].to_broadcast((P, SB, HPC, D))
                    s1 = sin_sb[:, :, None, 0:D2].to_broadcast((P, SB, HPC, D2))
                    s2 = sin_sb[:, :, None, D2:D].to_broadcast((P, SB, HPC, D2))
                    ca = ptmp.tile([P, SB, HPC, D], F32, name="ca", tag="ca", bufs=1)
                    th = ptmp.tile([P, SB, HPC, D2], F32, name="th", tag="th", bufs=1)
                    t2 = ptmp.tile([P, SB, HPC, D2], F32, name="t2", tag="t2", bufs=1)
                    rx = ptmp.tile([P, SB, HPC, D], BF, name="rx", tag="rx", bufs=1)
                    nc.vector.tensor_tensor(out=ca[:], in0=xv, in1=cb, op=ALU.mult)
                    nc.vector.tensor_tensor(out=th[:], in0=xv[:, :, :, D2:D], in1=s1, op=ALU.mult)
                    nc.vector.tensor_tensor(out=rx[:, :, :, 0:D2], in0=ca[:, :, :, 0:D2], in1=th[:], op=ALU.subtract)
                    nc.vector.tensor_tensor(out=t2[:], in0=xv[:, :, :, 0:D2], in1=s2, op=ALU.mult)
                    nc.vector.tensor_tensor(out=rx[:, :, :, D2:D], in0=ca[:, :, :, D2:D], in1=t2[:], op=ALU.add)
                    # transpose [s, d] -> [d, s] per (s-block, head)
                    for sb in range(SB):
                        for h in range(HPC):
                            pst = tpsum.tile([D, P], BF, name=f"pst{sb}{h}", tag="pst")
                            nc.tensor.transpose(pst[:], rx[:, sb, h], ident[:])
                            nc.vector.tensor_copy(
                                out=xTp[0:D, h, sb * P : (sb + 1) * P], in_=pst[:]
                            )

            # ---------------- phase A: attention --------------------------
            with tc.tile_pool(name="dram", bufs=1, space="DRAM") as dram:
                cc_in0 = dram.tile([P, S], BF)
                cc_out0 = dram.tile([4 * P, S], BF)
                cc_in1 = dram.tile([P, S], BF)
                cc_out1 = dram.tile([4 * P, S], BF)
                cc_ins = [cc_in0, cc_in1]
                cc_outs = [cc_out0, cc_out1]

                with tc.tile_pool(name="probs", bufs=28) as probspool, \
                     tc.tile_pool(name="spsum", bufs=3, space="PSUM") as spsum, \
                     tc.tile_pool(name="pvpsum", bufs=2, space="PSUM") as pvpsum, \
                     tc.tile_pool(name="atmp", bufs=4) as atmp:
                    for h in range(HPC):
                        probs_h = [
                            probspool.tile([P, S], BF, name=f"probs_{h}_{t}", tag="probs")
                            for t in range(SB)
                        ]
                        for t in range(SB):
                            lhs = kTp[:, h, t * P : (t + 1) * P]
                            for half in range(S // QK_N):
                                ssc = spsum.tile([P, QK_N], F32, name=f"ssc{h}{t}{half}", tag="ssc")
                                for q4 in range(QK_N // 512):
                                    nc.tensor.matmul(
                                        ssc[:, q4 * 512 : (q4 + 1) * 512],
                                        lhs,
                                        qTp[:, h, half * QK_N + q4 * 512 : half * QK_N + (q4 + 1) * 512],
                                        start=True,
                                        stop=True,
                                    )
                                nc.scalar.activation(
                                    probs_h[t][:, half * QK_N : (half + 1) * QK_N],
                                    ssc[:],
                                    ACTF.Exp,
                                )
                        # PV: V' stationary, probsT moving; psum rows = d + sums
                        for sc in range(NSC):
                            pvp = pvpsum.tile([DV, SC], F32, name=f"pvp{h}{sc}", tag="pvp")
                            for t in range(SB):
                                nc.tensor.matmul(
                                    pvp[:],
                                    Vp[:, t, h * DV : (h + 1) * DV],
                                    probs_h[t][:, sc * SC : (sc + 1) * SC],
                                    start=(t == 0),
                                    stop=(t == SB - 1),
                                )
                            # normalize: attnT[d, s] = pv[d, s] / sums[s]
                            rc = atmp.tile([P, SC], F32, name=f"rc{h}{sc}", tag="rc")
                            nc.vector.reciprocal(rc[D : D + 1, :], pvp[D : D + 1, :])
                            rb = atmp.tile([D, SC], F32, name=f"rb{h}{sc}", tag="rb")
                            nc.sync.dma_start(rb[:], rc[D : D + 1, None, :].to_broadcast((1, D, SC)))
                            nc.vector.tensor_tensor(
                                out=attnT[:, h, sc * SC : (sc + 1) * SC],
                                in0=pvp[0:D, :],
                                in1=rb[:],
                                op=ALU.mult,
                            )
                        # after each head pair completes, ship it
                        if h % 2 == 1:
                            i = h // 2
                            nc.sync.dma_start(
                                cc_ins[i][:].rearrange("(hh p) s -> p hh s", p=D),
                                attnT[:, h - 1 : h + 1, :],
                            )
                            nc.gpsimd.collective_compute(
                                "AllGather", ALU.bypass,
                                replica_groups=[[0, 1, 2, 3], [4, 5, 6, 7]],
                                ins=[cc_ins[i][:].opt()], outs=[cc_outs[i][:].opt()],
                            )

                # ---------------- phase O: output projection --------------
                # cc_out[i] rows: quad rank g's head pair i -> global o-chunk 2g+i
                with tc.tile_pool(name="opool", bufs=1) as opool, \
                     tc.tile_pool(name="opsum", bufs=4, space="PSUM") as opsum, \
                     tc.tile_pool(name="otmp", bufs=3) as otmp:
                    aT = opool.tile([P, 2, 4, S], BF)   # [p, pair, quadrank, s]
                    nc.sync.dma_start(aT[:, 0], cc_outs[0][:].rearrange("(g p) s -> p g s", p=P))
                    nc.sync.dma_start(aT[:, 1], cc_outs[1][:].rearrange("(g p) s -> p g s", p=P))
                    owT_sb = opool.tile([P, KC, OC], BF)
                    nc.sync.dma_start(owT_sb[:], owT[:].rearrange("(a p) o -> p a o", p=P))
                    for sb in range(SB):
                        pso = opsum.tile([P, OC], F32, name=f"pso{sb}", tag="pso")
                        for kc in range(KC):
                            g, pair = kc // 2, kc % 2
                            nc.tensor.matmul(
                                pso[:],
                                aT[:, pair, g, sb * P : (sb + 1) * P],
                                owT_sb[:, kc],
                                start=(kc == 0),
                                stop=(kc == KC - 1),
                            )
                        of = otmp.tile([P, OC], F32, name=f"of{sb}", tag="of")
                        nc.vector.tensor_tensor(out=of[:], in0=pso[:], in1=ob_sb[:], op=ALU.add)
                        nc.sync.dma_start(out[sb * P : (sb + 1) * P, :], of[:])

    nc.finalize()
    return nc


_NC_CACHE = None


def _get_nc():
    global _NC_CACHE
    if _NC_CACHE is None:
        _NC_CACHE = build_nc()
    return _NC_CACHE


def _prep_in_maps(inputs):
    bf16 = ml_dtypes.bfloat16
    hidden = np.asarray(inputs["hidden_states"], np.float32)
    cos = np.ascontiguousarray(np.asarray(inputs["cos"], np.float32))
    sin = np.ascontiguousarray(np.asarray(inputs["sin"], np.float32))
    q_w = np.asarray(inputs["q_w"], np.float32)
    q_b = np.asarray(inputs["q_b"], np.float32)
    kv_w = np.asarray(inputs["kv_w"], np.float32)
    kv_b = np.asarray(inputs["kv_b"], np.float32)
    o_w = np.asarray(inputs["o_w"], np.float32)
    o_b = np.asarray(inputs["o_b"], np.float32)

    hT = [np.ascontiguousarray(hidden[b].T).astype(bf16) for b in range(B)]

    in_maps = []
    for c in range(NCORES):
        b, hg = divmod(c, 4)
        sl = slice(hg * OC, (hg + 1) * OC)
        vsl = slice(H + hg * OC, H + (hg + 1) * OC)
        in_maps.append({
            "hT": hT[b],
            "qwT": np.ascontiguousarray(q_w[sl].T).astype(bf16),
            "kwT": np.ascontiguousarray(kv_w[sl].T).astype(bf16),
            "vwT": np.ascontiguousarray(kv_w[vsl].T).astype(bf16),
            "owT": np.ascontiguousarray(o_w[sl].T).astype(bf16),
            "qb": np.ascontiguousarray(np.broadcast_to(q_b[sl], (P, OC))),
            "kb": np.ascontiguousarray(np.broadcast_to(kv_b[sl], (P, OC))),
            "vb": np.ascontiguousarray(np.broadcast_to(kv_b[vsl], (P, OC))),
            "ob": np.ascontiguousarray(np.broadcast_to(o_b[sl], (P, OC))),
            "cosd": cos,
            "sind": sin,
        })
    return in_maps


def _assemble(results):
    out = np.empty((B, S, H), np.float32)
    for c in range(NCORES):
        b, hg = divmod(c, 4)
        out[b, :, hg * OC : (hg + 1) * OC] = results[c]["out"]
    return out


def kernel(**inputs):
    from concourse.bass_utils import run_bass_kernel_spmd

    nc = _get_nc()
    in_maps = _prep_in_maps(inputs)
    res = run_bass_kernel_spmd(nc, in_maps, list(range(NCORES)))
    results = res.results if hasattr(res, "results") else res
    return _assemble(results)


# revision 16
# speedup vs baseline: 1.0118x; 1.0118x over previous
"""Trainium2 Bass kernel for fused attention block (B=2, S=2048, H=1024, N=16, D=64).

Sharding: 8 cores = 2 batches (DP) x 4 head-groups (TP, 4 heads each).
Per core: q/kv projections + LN + RoPE + attention for its 4 heads, AllGather
of normalized attention outputs (bf16) within the batch quad (split in two so
the first gather overlaps attention), then a 256-column slice of the output
projection.

Pipeline: Q path runs first (proj -> LN/rope -> DMA-transpose), then the KV
projection streams on PE while attention (ACT-bound exp) consumes per-head
K tiles as they become ready. PV keeps V' (with an extra ones column for the
softmax sums) stationary so probs tiles die immediately after each t-block.
"""

import numpy as np
import ml_dtypes

import concourse.bass as bass
from concourse import bacc
import concourse.mybir as mybir
import concourse.tile as tile

# problem shape (hardcoded per contract)
B, S, H, NH, D = 2, 2048, 1024, 16, 64
EPS = 1.0 / 65530.0
NCORES = 8
HPC = 4            # heads per core
OC = HPC * D       # 256 head-dims per core
P = 128
SB = S // P        # 16 s-blocks
KC = H // P        # 8 contraction chunks of 128
D2 = D // 2
SCALE = 1.0 / 8.0  # 1/sqrt(D)
DV = D + 1         # V columns per head incl. ones column
SC = 512           # s-chunk for PV accumulation
NSC = S // SC      # 4

BF = mybir.dt.bfloat16
F32 = mybir.dt.float32
ALU = mybir.AluOpType
ACTF = mybir.ActivationFunctionType


def build_nc():
    nc = bacc.Bacc(num_devices=NCORES)

    hT = nc.declare_dram_parameter("hT", [H, S], BF, isOutput=False)
    qwT = nc.declare_dram_parameter("qwT", [H, OC], BF, isOutput=False)
    kwT = nc.declare_dram_parameter("kwT", [H, OC], BF, isOutput=False)
    vwT = nc.declare_dram_parameter("vwT", [H, OC], BF, isOutput=False)
    owT = nc.declare_dram_parameter("owT", [H, OC], BF, isOutput=False)
    qb = nc.declare_dram_parameter("qb", [P, OC], F32, isOutput=False)
    kb = nc.declare_dram_parameter("kb", [P, OC], F32, isOutput=False)
    vb = nc.declare_dram_parameter("vb", [P, OC], F32, isOutput=False)
    ob = nc.declare_dram_parameter("ob", [P, OC], F32, isOutput=False)
    cosd = nc.declare_dram_parameter("cosd", [S, D], F32, isOutput=False)
    sind = nc.declare_dram_parameter("sind", [S, D], F32, isOutput=False)
    out = nc.declare_dram_parameter("out", [S, OC], F32, isOutput=True)

    with tile.TileContext(nc) as tc:
        with tc.tile_pool(name="persist", bufs=1) as persist:
            cos_sb = persist.tile([P, SB, D], F32)
            nc.sync.dma_start(cos_sb[:], cosd[:].rearrange("(a p) d -> p a d", p=P))
            sin_sb = persist.tile([P, SB, D], F32)
            nc.sync.dma_start(sin_sb[:], sind[:].rearrange("(a p) d -> p a d", p=P))
            qb_sb = persist.tile([P, OC], F32)
            nc.sync.dma_start(qb_sb[:], qb[:])
            kb_sb = persist.tile([P, OC], F32)
            nc.sync.dma_start(kb_sb[:], kb[:])
            vb_sb = persist.tile([P, OC], F32)
            nc.sync.dma_start(vb_sb[:], vb[:])
            ob_sb = persist.tile([P, OC], F32)
            nc.sync.dma_start(ob_sb[:], ob[:])

            # transposed q/k in head-pair chunks: chunk c rows 0..63 = head 2c,
            # rows 64..127 = head 2c+1 (k=64 matmuls slice these)
            qT2 = persist.tile([P, 2, S], BF)
            kT2 = persist.tile([P, 2, S], BF)
            # v in [s, head*(D+1)] layout: D data cols + 1 ones col per head
            Vp = persist.tile([P, SB, HPC * DV], BF)
            for h in range(HPC):
                nc.gpsimd.memset(Vp[:, :, h * DV + D : (h + 1) * DV], 1.0)
            attnT = persist.tile([D, HPC, S], BF)      # normalized [d, h, s]

            mu_q = persist.tile([P, SB, HPC], F32)
            mu_k = persist.tile([P, SB, HPC], F32)
            var_q = persist.tile([P, SB, HPC], F32)
            var_k = persist.tile([P, SB, HPC], F32)
            rstd_q = persist.tile([P, SB, HPC], F32)
            rstd_k = persist.tile([P, SB, HPC], F32)
            std_q = persist.tile([P, SB, HPC], F32)
            std_k = persist.tile([P, SB, HPC], F32)
            eps_t = persist.tile([P, 1], F32)
            nc.gpsimd.memset(eps_t[:], EPS)

            def stats(xf, sb, mu, var, pool):
                xv = xf[:, sb].rearrange("p (h d) -> p h d", h=HPC)
                sq = pool.tile([P, HPC, D], F32, name=f"sq{sb}", tag="sq")
                nc.vector.tensor_tensor(out=sq[:], in0=xv, in1=xv, op=ALU.mult)
                nc.vector.tensor_reduce(
                    out=mu[:, sb], in_=xv, axis=mybir.AxisListType.X, op=ALU.add
                )
                nc.vector.tensor_reduce(
                    out=var[:, sb], in_=sq[:], axis=mybir.AxisListType.X, op=ALU.add
                )
                nc.vector.tensor_scalar_mul(mu[:, sb], mu[:, sb], 1.0 / D)
                nc.vector.tensor_scalar_mul(var[:, sb], var[:, sb], 1.0 / D)
                mu2 = pool.tile([P, HPC], F32, name=f"mu2{sb}", tag="mu2")
                nc.vector.tensor_tensor(out=mu2[:], in0=mu[:, sb], in1=mu[:, sb], op=ALU.mult)
                nc.vector.tensor_tensor(out=var[:, sb], in0=var[:, sb], in1=mu2[:], op=ALU.subtract)

            def ln_rope_transpose(xf, mu, rstd, xT2, pool):
                """Batched LN apply + rope over [P, SB, HPC, D], then
                DMA-transpose head-pair blocks into xT2."""
                xv = xf[:].rearrange("p s (h d) -> p s h d", h=HPC)
                mu_b = mu[:, :, :, None].to_broadcast((P, SB, HPC, D))
                rs_b = rstd[:, :, :, None].to_broadcast((P, SB, HPC, D))
                nc.vector.tensor_tensor(out=xv, in0=xv, in1=mu_b, op=ALU.subtract)
                nc.vector.tensor_tensor(out=xv, in0=xv, in1=rs_b, op=ALU.mult)
                cb = cos_sb[:, :, None, :].to_broadcast((P, SB, HPC, D))
                s1 = sin_sb[:, :, None, 0:D2].to_broadcast((P, SB, HPC, D2))
                s2 = sin_sb[:, :, None, D2:D].to_broadcast((P, SB, HPC, D2))
                ca = pool.tile([P, SB, HPC, D], F32, name="ca", tag="ca", bufs=1)
                th = pool.tile([P, SB, HPC, D2], F32, name="th", tag="th", bufs=1)
                t2 = pool.tile([P, SB, HPC, D2], F32, name="t2", tag="t2", bufs=1)
                rx = pool.tile([P, SB, HPC, D], BF, name="rx", tag="rx", bufs=1)
                nc.vector.tensor_tensor(out=ca[:], in0=xv, in1=cb, op=ALU.mult)
                nc.vector.tensor_tensor(out=th[:], in0=xv[:, :, :, D2:D], in1=s1, op=ALU.mult)
                nc.vector.tensor_tensor(out=rx[:, :, :, 0:D2], in0=ca[:, :, :, 0:D2], in1=th[:], op=ALU.subtract)
                nc.vector.tensor_tensor(out=t2[:], in0=xv[:, :, :, 0:D2], in1=s2, op=ALU.mult)
                nc.vector.tensor_tensor(out=rx[:, :, :, D2:D], in0=ca[:, :, :, D2:D], in1=t2[:], op=ALU.add)
                rx2 = rx[:].rearrange("p s h d -> p s (h d)")
                for sb in range(SB):
                    for c in range(2):
                        nc.sync.dma_start(
                            xT2[:, c, sb * P : (sb + 1) * P],
                            rx2[:, sb, c * P : (c + 1) * P],
                            transpose=True,
                        )

            # ---------------- phase Q ----------------------------------
            with tc.tile_pool(name="pw", bufs=1) as pw, \
                 tc.tile_pool(name="projpsum", bufs=2, space="PSUM") as projpsum, \
                 tc.tile_pool(name="ptmp", bufs=3) as ptmp:
                hT_sb = pw.tile([P, KC, S], BF)
                nc.sync.dma_start(hT_sb[:], hT[:].rearrange("(a p) s -> p a s", p=P))
                qwT_sb = pw.tile([P, KC, OC], BF)
                nc.sync.dma_start(qwT_sb[:], qwT[:].rearrange("(a p) o -> p a o", p=P))
                kvwT_sb = pw.tile([P, KC, 2 * OC], BF)
                nc.sync.dma_start(kvwT_sb[:, :, 0:OC], kwT[:].rearrange("(a p) o -> p a o", p=P))
                nc.sync.dma_start(kvwT_sb[:, :, OC : 2 * OC], vwT[:].rearrange("(a p) o -> p a o", p=P))
                qf = pw.tile([P, SB, OC], F32)
                kf = pw.tile([P, SB, OC], F32)

                for sb in range(SB):
                    pq = projpsum.tile([P, OC], F32, name=f"pq{sb}", tag="pq")
                    for kc in range(KC):
                        nc.tensor.matmul(
                            pq[:], hT_sb[:, kc, sb * P : (sb + 1) * P], qwT_sb[:, kc],
                            start=(kc == 0), stop=(kc == KC - 1),
                        )
                    nc.vector.tensor_tensor(out=qf[:, sb], in0=pq[:], in1=qb_sb[:], op=ALU.add)
                    stats(qf, sb, mu_q, var_q, ptmp)
                nc.scalar.activation(std_q[:], var_q[:], ACTF.Sqrt, bias=eps_t[:])
                nc.vector.reciprocal(rstd_q[:], std_q[:])
                nc.vector.tensor_scalar_mul(rstd_q[:], rstd_q[:], SCALE)
                ln_rope_transpose(qf, mu_q, rstd_q, qT2, ptmp)

                # ---------------- phase KV ------------------------------
                for sb in range(SB):
                    pkv = projpsum.tile([P, 2 * OC], F32, name=f"pkv{sb}", tag="pkv")
                    for kc in range(KC):
                        nc.tensor.matmul(
                            pkv[:], hT_sb[:, kc, sb * P : (sb + 1) * P], kvwT_sb[:, kc],
                            start=(kc == 0), stop=(kc == KC - 1),
                        )
                    nc.vector.tensor_tensor(out=kf[:, sb], in0=pkv[:, 0:OC], in1=kb_sb[:], op=ALU.add)
                    nc.vector.tensor_tensor(
                        out=Vp[:, sb].rearrange("p (h e) -> p h e", h=HPC)[:, :, 0:D],
                        in0=pkv[:, OC : 2 * OC].rearrange("p (h d) -> p h d", h=HPC),
                        in1=vb_sb[:].rearrange("p (h d) -> p h d", h=HPC),
                        op=ALU.add,
                    )
                    stats(kf, sb, mu_k, var_k, ptmp)
                nc.scalar.activation(std_k[:], var_k[:], ACTF.Sqrt, bias=eps_t[:])
                nc.vector.reciprocal(rstd_k[:], std_k[:])
                ln_rope_transpose(kf, mu_k, rstd_k, kT2, ptmp)

            # ---------------- phase A: attention ------------------------
            with tc.tile_pool(name="dram", bufs=1, space="DRAM") as dram:
                cc_in0 = dram.tile([P, S], BF)
                cc_out0 = dram.tile([4 * P, S], BF)
                cc_in1 = dram.tile([P, S], BF)
                cc_out1 = dram.tile([4 * P, S], BF)
                cc_ins = [cc_in0, cc_in1]
                cc_outs = [cc_out0, cc_out1]

                with tc.tile_pool(name="probs", bufs=8) as probspool, \
                     tc.tile_pool(name="spsum", bufs=2, space="PSUM") as spsum, \
                     tc.tile_pool(name="pvpsum", bufs=1, space="PSUM") as pvpsum, \
                     tc.tile_pool(name="atmp", bufs=6) as atmp:
                    for h in range(HPC):
                        ch, ro = h // 2, (h % 2) * D
                        pvp = pvpsum.tile([DV, S], F32, name=f"pvp{h}", tag="pvp")
                        for t in range(SB):
                            probs_t = probspool.tile([P, S], BF, name=f"probs_{h}_{t}", tag="probs")
                            lhs = kT2[ro : ro + D, ch, t * P : (t + 1) * P]
                            for half in range(2):
                                ssc = spsum.tile([P, S // 2], F32, name=f"ssc{h}{t}{half}", tag="ssc")
                                for q4 in range(2):
                                    o0 = half * 1024 + q4 * 512
                                    nc.tensor.matmul(
                                        ssc[:, q4 * 512 : (q4 + 1) * 512],
                                        lhs,
                                        qT2[ro : ro + D, ch, o0 : o0 + 512],
                                        start=True, stop=True,
                                    )
                                nc.scalar.activation(
                                    probs_t[:, half * 1024 : (half + 1) * 1024],
                                    ssc[:], ACTF.Exp,
                                )
                            for sc in range(NSC):
                                nc.tensor.matmul(
                                    pvp[:, sc * SC : (sc + 1) * SC],
                                    Vp[:, t, h * DV : (h + 1) * DV],
                                    probs_t[:, sc * SC : (sc + 1) * SC],
                                    start=(t == 0), stop=(t == SB - 1),
                                )
                        # normalize: attnT[d, s] = pv[d, s] * (1 / sums[s])
                        for sc in range(NSC):
                            rc = atmp.tile([P, SC], F32, name=f"rc{h}{sc}", tag="rc")
                            nc.vector.reciprocal(rc[D : D + 1, :], pvp[D : D + 1, sc * SC : (sc + 1) * SC])
                            rb = atmp.tile([D, SC], F32, name=f"rb{h}{sc}", tag="rb")
                            nc.gpsimd.dma_start(rb[:], rc[D : D + 1, None, :].to_broadcast((1, D, SC)))
                            nc.vector.tensor_tensor(
                                out=attnT[:, h, sc * SC : (sc + 1) * SC],
                                in0=pvp[0:D, sc * SC : (sc + 1) * SC],
                                in1=rb[:],
                                op=ALU.mult,
                            )
                        # ship each completed head pair
                        if h % 2 == 1:
                            i = h // 2
                            nc.gpsimd.dma_start(
                                cc_ins[i][:].rearrange("(hh p) s -> p hh s", p=D),
                                attnT[:, h - 1 : h + 1, :],
                            )
                            nc.gpsimd.collective_compute(
                                "AllGather", ALU.bypass,
                                replica_groups=[[0, 1, 2, 3], [4, 5, 6, 7]],
                                ins=[cc_ins[i][:].opt()], outs=[cc_outs[i][:].opt()],
                            )

                # ---------------- phase O: output projection ------------
                # cc_out[i] rows: quad rank g's head pair i -> global o-chunk 2g+i
                with tc.tile_pool(name="opool", bufs=1) as opool, \
                     tc.tile_pool(name="opsum", bufs=4, space="PSUM") as opsum, \
                     tc.tile_pool(name="otmp", bufs=3) as otmp:
                    aT = opool.tile([P, 2, 4, S], BF)   # [p, pair, quadrank, s]
                    nc.sync.dma_start(aT[:, 0], cc_outs[0][:].rearrange("(g p) s -> p g s", p=P))
                    nc.sync.dma_start(aT[:, 1], cc_outs[1][:].rearrange("(g p) s -> p g s", p=P))
                    owT_sb = opool.tile([P, KC, OC], BF)
                    nc.sync.dma_start(owT_sb[:], owT[:].rearrange("(a p) o -> p a o", p=P))
                    for sb in range(SB):
                        pso = opsum.tile([P, OC], F32, name=f"pso{sb}", tag="pso")
                        for kc in range(KC):
                            g, pair = kc // 2, kc % 2
                            nc.tensor.matmul(
                                pso[:],
                                aT[:, pair, g, sb * P : (sb + 1) * P],
                                owT_sb[:, kc],
                                start=(kc == 0), stop=(kc == KC - 1),
                            )
                        of = otmp.tile([P, OC], F32, name=f"of{sb}", tag="of")
                        nc.vector.tensor_tensor(out=of[:], in0=pso[:], in1=ob_sb[:], op=ALU.add)
                        nc.sync.dma_start(out[sb * P : (sb + 1) * P, :], of[:])

    nc.finalize()
    return nc


_NC_CACHE = None


def _get_nc():
    global _NC_CACHE
    if _NC_CACHE is None:
        _NC_CACHE = build_nc()
    return _NC_CACHE


def _prep_in_maps(inputs):
    bf16 = ml_dtypes.bfloat16
    hidden = np.asarray(inputs["hidden_states"], np.float32)
    cos = np.ascontiguousarray(np.asarray(inputs["cos"], np.float32))
    sin = np.ascontiguousarray(np.asarray(inputs["sin"], np.float32))
    q_w = np.asarray(inputs["q_w"], np.float32)
    q_b = np.asarray(inputs["q_b"], np.float32)
    kv_w = np.asarray(inputs["kv_w"], np.float32)
    kv_b = np.asarray(inputs["kv_b"], np.float32)
    o_w = np.asarray(inputs["o_w"], np.float32)
    o_b = np.asarray(inputs["o_b"], np.float32)

    hT = [np.ascontiguousarray(hidden[b].T).astype(bf16) for b in range(B)]

    in_maps = []
    for c in range(NCORES):
        b, hg = divmod(c, 4)
        sl = slice(hg * OC, (hg + 1) * OC)
        vsl = slice(H + hg * OC, H + (hg + 1) * OC)
        in_maps.append({
            "hT": hT[b],
            "qwT": np.ascontiguousarray(q_w[sl].T).astype(bf16),
            "kwT": np.ascontiguousarray(kv_w[sl].T).astype(bf16),
            "vwT": np.ascontiguousarray(kv_w[vsl].T).astype(bf16),
            "owT": np.ascontiguousarray(o_w[sl].T).astype(bf16),
            "qb": np.ascontiguousarray(np.broadcast_to(q_b[sl], (P, OC))),
            "kb": np.ascontiguousarray(np.broadcast_to(kv_b[sl], (P, OC))),
            "vb": np.ascontiguousarray(np.broadcast_to(kv_b[vsl], (P, OC))),
            "ob": np.ascontiguousarray(np.broadcast_to(o_b[sl], (P, OC))),
            "cosd": cos,
            "sind": sin,
        })
    return in_maps


def _assemble(results):
    out = np.empty((B, S, H), np.float32)
    for c in range(NCORES):
        b, hg = divmod(c, 4)
        out[b, :, hg * OC : (hg + 1) * OC] = results[c]["out"]
    return out


def kernel(**inputs):
    from concourse.bass_utils import run_bass_kernel_spmd

    nc = _get_nc()
    in_maps = _prep_in_maps(inputs)
    res = run_bass_kernel_spmd(nc, in_maps, list(range(NCORES)))
    results = res.results if hasattr(res, "results") else res
    return _assemble(results)
